# revision 19
# baseline (speedup 1.0000x reference)
"""Causal self-attention (B=4, L=2048, D=1024, H=16) on 8 Trainium2 NeuronCores.

Sharding: core c handles batch b = c//2 and head-group hg = c%2 (8 of 16 heads).
Each core computes its local QKV projection, causal flash-style attention for
its 8 heads, and a partial output projection against its 512 columns of
proj_w. The host sums the two partial outputs per batch and adds proj_b.

Device layouts (per core):
  xT    [1024, L]   x[b].T              (contraction dim d on partitions)
  w_qk  [1024,1024] qkv_w local q+k rows, transposed; q part pre-scaled by
                    HEAD_DIM**-0.5 (folded into weights+bias); fully
                    resident in SBUF
  qkTb  [128,512]x(8,4)  (q;k) features on partitions, one tile per l-block
  vaug  [L, 8*65]   v in token-major layout, one extra ones-column per head
                    (the ones column makes the PV matmul also produce the
                    softmax denominator as psum row 64)
  yTb   [128,512]x(4,4)  normalized attention output, per l-block
  projT [512, 1024] proj_w local columns, transposed

Softmax skips the max-subtraction (scores are O(+-10) here, far from fp32
overflow) so P = exp(S).

Perf structure (the attention inner loop is exp-paced on the Scalar engine;
everything else must hide under it):
  - diagonal ki-tiles computed at fine granularity: for ki = 4*qj+j the
    score matmul covers only q in [128j, 512), so scores/exp/PV shrink by
    ~25% on the diagonal and masking reduces to one [128,128] triangle
    multiply per tile (exact-causal, no wasted exp on masked halves)
  - QKV of l-block lb+1 and proj of lb-1 are emitted in chunks inside
    attention(lb)'s head loop so their PE work fills the exp-bound stretch
  - reciprocal batched to one [8,512] op per l-block (DVE iterative divide
    is ~6.4 cyc/elem, so per-head [1,512] reciprocals were ~3.3us each)
  - denominator rows cross partitions via DMA (engines can't address
    single partitions off 32-alignment)
"""

import os
import sys

import numpy as np

for _p in ("/opt/trn_rl_repo", "/root/.axon_site/_ro/trn_rl_repo"):
    if os.path.isdir(_p) and _p not in sys.path:
        sys.path.append(_p)

import ml_dtypes  # noqa: E402
import concourse.bass as bass  # noqa: E402
import concourse.tile as tile  # noqa: E402
from concourse import bacc, mybir  # noqa: E402
from concourse.bass_utils import run_bass_kernel_spmd  # noqa: E402

DIM = 1024
NUM_HEADS = 16
HEAD_DIM = 64
SCALE = HEAD_DIM**-0.5
B = 4
L = 2048
NCORES = 8
HLOC = 8  # heads per core

F32 = mybir.dt.float32
BF16 = mybir.dt.bfloat16

DT_IN = BF16   # x / weights
DT_QK = BF16   # q/k storage
DT_PV = BF16   # P / v_aug
DT_PROJ = BF16  # yT / projT

NP_IN = ml_dtypes.bfloat16


def schedule_from_mask(am, Lc):
    """Classify [128k x 512q] score blocks from attn_mask[q, k].

    Returns (sched, patterns): sched[qj] = list of (ki, pat_idx|None) blocks
    to compute; patterns = list of [128, 512] float32 0/1 arrays (k on
    partitions, q on free dim) for partially-masked blocks.
    """
    am = np.asarray(am) != 0
    sched, patterns, pat_ids = [], [], {}
    for qj in range(Lc // 512):
        row = []
        for ki in range(Lc // 128):
            blk = am[qj * 512:(qj + 1) * 512, ki * 128:(ki + 1) * 128]  # [q,k]
            if not blk.any():
                continue
            if blk.all():
                row.append((ki, None))
                continue
            pat = np.ascontiguousarray(blk.T).astype(np.float32)  # [k,q]
            key = pat.tobytes()
            if key not in pat_ids:
                pat_ids[key] = len(patterns)
                patterns.append(pat)
            row.append((ki, pat_ids[key]))
        sched.append(row)
    return sched, patterns


def verify_causal(sched, patterns, Lc):
    """Check the mask is exactly causal-tril at block granularity (the
    fine-grained diagonal codegen assumes it)."""
    k = np.arange(128)[:, None]
    for qj in range(Lc // 512):
        blocks = sched[qj]
        full = [ki for ki, pat in blocks if pat is None]
        diag = [(ki, pat) for ki, pat in blocks if pat is not None]
        if full != list(range(4 * qj)) or len(diag) != 4:
            return False
        for ki, pat in diag:
            j = ki - 4 * qj
            if j not in range(4):
                return False
            q = np.arange(512)[None, :]
            expect = (q >= 128 * j + k).astype(np.float32)
            if not np.array_equal(patterns[pat], expect):
                return False
    return True


def build_nc(Lc, sched, n_pat=0, cfg=None, nrep=1, phases=3):
    """Emit the per-core Bass/Tile program. Same program runs on all cores."""
    NLB = Lc // 512   # l-blocks (also q-blocks)
    NKT = Lc // 128   # k-tiles
    ND = DIM // 128   # contraction tiles for QKV

    nc = bacc.Bacc("TRN2", target_bir_lowering=False, debug=False)

    xT = nc.dram_tensor("xT", [DIM, Lc], DT_IN, kind="ExternalInput")
    w_qk = nc.dram_tensor("w_qk", [DIM, 1024], DT_IN, kind="ExternalInput")
    w_v = nc.dram_tensor("w_v", [DIM, 512], DT_IN, kind="ExternalInput")
    bqk = nc.dram_tensor("bqk", [128, 8], F32, kind="ExternalInput")
    bv = nc.dram_tensor("bv", [1, 512], DT_IN, kind="ExternalInput")
    tri_d = nc.dram_tensor("tri", [128, 128], DT_PV, kind="ExternalInput")
    projT = nc.dram_tensor("projT", [512, 1024], DT_PROJ, kind="ExternalInput")
    y = nc.dram_tensor("y", [Lc, 1024], F32, kind="ExternalOutput")

    with tile.TileContext(nc) as tc:
        import contextlib
        with contextlib.ExitStack() as ctx:
            sing = ctx.enter_context(tc.tile_pool(name="sing", bufs=1))

            qkTb = [[sing.tile([128, 512], DT_QK, tag=f"qkT{e}_{lb}",
                               name=f"qkT{e}_{lb}") for lb in range(NLB)]
                    for e in range(8)]
            vaug = [sing.tile([128, HLOC * 65], DT_PV, tag=f"vaug{t}",
                              name=f"vaug{t}") for t in range(NKT)]
            yTb = [[sing.tile([128, 512], DT_PROJ, tag=f"yT{f}_{lb}",
                              name=f"yT{f}_{lb}") for lb in range(NLB)]
                   for f in range(4)]
            projT_sb = [sing.tile([128, 1024], DT_PROJ, tag=f"pw{t}",
                                  name=f"pw{t}") for t in range(4)]
            wv_sb = [sing.tile([128, 512], DT_IN, tag=f"wv{t}", name=f"wv{t}")
                     for t in range(ND)]
            wqk_sb = [sing.tile([128, 1024], DT_IN, tag=f"wqk{t}",
                                name=f"wqk{t}") for t in range(ND)]
            bqk_sb = sing.tile([128, 8], F32, tag="bqk_sb", name="bqk_sb")
            bv_sb = sing.tile([1, 512], DT_IN, tag="bv_sb", name="bv_sb")
            tri_sb = sing.tile([128, 128], DT_PV, tag="tri", name="tri")
            ones_col = sing.tile([1, 128], DT_IN, tag="ones_col",
                                 name="ones_col")

            nc.vector.memset(ones_col[:, :], 1.0)
            for t in range(NKT):
                va = vaug[t].rearrange("p (h c) -> p h c", c=65)
                nc.vector.memset(va[:, :, 64:65], 1.0)

            # weights first: the first QKV matmuls gate on these
            for t in range(ND):
                nc.sync.dma_start(wqk_sb[t][:, :],
                                  w_qk[t * 128:(t + 1) * 128, :])
            for t in range(ND):
                nc.sync.dma_start(wv_sb[t][:, :], w_v[t * 128:(t + 1) * 128, :])
            nc.sync.dma_start(bqk_sb[:, :], bqk[:, :])
            nc.sync.dma_start(bv_sb[:, :], bv[:, :])
            nc.sync.dma_start(tri_sb[:, :], tri_d[:, :])
            for t in range(4):
                nc.sync.dma_start(projT_sb[t][:, :],
                                  projT[t * 128:(t + 1) * 128, :])

            xp = ctx.enter_context(tc.tile_pool(name="xp", bufs=18))
            ptp = ctx.enter_context(tc.tile_pool(name="ptp", bufs=8))
            osp = ctx.enter_context(tc.tile_pool(name="osp", bufs=14))
            dnp = ctx.enter_context(tc.tile_pool(name="dnp", bufs=2))
            inp = ctx.enter_context(tc.tile_pool(name="inp", bufs=2))
            invp = ctx.enter_context(tc.tile_pool(name="invp", bufs=6))
            repp = ctx.enter_context(tc.tile_pool(name="repp", bufs=4))
            outp = ctx.enter_context(tc.tile_pool(name="outp", bufs=4))
            pmm = ctx.enter_context(tc.tile_pool(name="pmm", bufs=3,
                                                 space="PSUM"))
            pov = ctx.enter_context(tc.tile_pool(name="pov", bufs=2,
                                                 space="PSUM"))

            def emit_x_dma(lb):
                xt = []
                for d in range(ND):
                    xd = xp.tile([128, 512], DT_IN, tag="xd",
                                 name=f"x{lb}_{d}")
                    nc.sync.dma_start(
                        xd[:, :],
                        xT[d * 128:(d + 1) * 128, lb * 512:lb * 512 + 512])
                    xt.append(xd)
                return xt

            def emit_qkv_qk(lb, e, xt):
                ps = pmm.tile([128, 512], F32, tag="ps", name=f"q{lb}_{e}")
                for d in range(ND):
                    nc.tensor.matmul(
                        ps[:, :],
                        lhsT=wqk_sb[d][:, e * 128:(e + 1) * 128],
                        rhs=xt[d][:, :],
                        start=(d == 0), stop=(d == ND - 1))
                nc.vector.tensor_scalar_add(
                    out=qkTb[e][lb][:, :], in0=ps[:, :],
                    scalar1=bqk_sb[:, e:e + 1])

            def emit_qkv_v(lb, ls, xt):
                lt = lb * 4 + ls
                ps = pmm.tile([128, 512], F32, tag="ps", name=f"v{lt}")
                for d in range(ND):
                    nc.tensor.matmul(
                        ps[:, :],
                        lhsT=xt[d][:, ls * 128:(ls + 1) * 128],
                        rhs=wv_sb[d][:, :],
                        start=(d == 0), stop=False)
                nc.tensor.matmul(
                    ps[:, :], lhsT=ones_col[:, :],
                    rhs=bv_sb[:, :], start=False, stop=True)
                dst = vaug[lt].rearrange("p (h c) -> p h c", c=65)[:, :, 0:64]
                src = ps.rearrange("p (h c) -> p h c", c=64)
                nc.vector.tensor_copy(dst, src)

            def emit_proj(qj, e2, ls):
                lr = qj * 512 + ls * 128
                ps = pmm.tile([128, 512], F32, tag="ps",
                              name=f"pj{qj}_{e2}_{ls}")
                for f in range(4):
                    nc.tensor.matmul(
                        ps[:, :],
                        lhsT=yTb[f][qj][:, ls * 128:(ls + 1) * 128],
                        rhs=projT_sb[f][:, e2 * 512:(e2 + 1) * 512],
                        start=(f == 0), stop=(f == 3))
                ob = outp.tile([128, 512], F32, tag="ob",
                               name=f"yo{qj}_{e2}_{ls}")
                nc.vector.tensor_copy(ob[:, :], ps[:, :])
                nc.sync.dma_start(
                    y[lr:lr + 128, e2 * 512:(e2 + 1) * 512], ob[:, :])

            PROJ_CHUNKS = [[(0, 0), (0, 1)], [(0, 2), (0, 3)],
                           [(1, 0), (1, 1)], [(1, 2), (1, 3)]]

            _ENGS = (mybir.EngineType.PE, mybir.EngineType.Activation,
                     mybir.EngineType.DVE, mybir.EngineType.Pool,
                     mybir.EngineType.SP)
            rep_ctx = (tc.For_i(0, nrep, 1, hint_engines=_ENGS) if nrep > 1
                       else contextlib.nullcontext())
            with rep_ctx:
                # l-block 0's QKV runs up front; later blocks are emitted in
                # chunks inside the previous block's attention
                xt_cur = emit_x_dma(0)
                for e in range(8):
                    emit_qkv_qk(0, e, xt_cur)
                for ls in range(4):
                    emit_qkv_v(0, ls, xt_cur)

                for qj in range(NLB):
                    if phases < 2:
                        break
                    xt_next = emit_x_dma(qj + 1) if qj + 1 < NLB else None
                    blocks = sched[qj]
                    full = [ki for ki, pat in blocks if pat is None]
                    fpairs = [full[i:i + 2] for i in range(0, len(full), 2)]
                    den8 = dnp.tile([8, 512], F32, tag="den8", name=f"dn{qj}")
                    osb_h = {}
                    # next l-block's QKV, dripped one psum tile at a time
                    # into the exp-paced attention stream (a burst of QKV
                    # matmuls here starves the Scalar engine; one ~2us tile
                    # per pair-iteration slots under the exp lookahead)
                    pending = []
                    if xt_next is not None:
                        pending = ([("qk", e) for e in range(8)]
                                   + [("v", ls) for ls in range(4)])

                    def drop_qkv(nmax=1):
                        for _ in range(nmax):
                            if not pending:
                                return
                            kind, idx = pending.pop(0)
                            if kind == "qk":
                                emit_qkv_qk(qj + 1, idx, xt_next)
                            else:
                                emit_qkv_v(qj + 1, idx, xt_next)
                    for hp in range(HLOC // 2):
                        t = hp
                        heads = (2 * hp, 2 * hp + 1)
                        po = {}
                        for h in heads:
                            po[h] = pov.tile([65, 512], F32, tag="po",
                                             name=f"o{qj}_{h}")
                        npv = {h: 0 for h in heads}
                        n_pv_total = len(full) + 4

                        def pv(h, ki, rhs_ap, q0=0):
                            nc.tensor.matmul(
                                po[h][:, q0:512],
                                lhsT=vaug[ki][:, h * 65:(h + 1) * 65],
                                rhs=rhs_ap,
                                start=(npv[h] == 0),
                                stop=(npv[h] == n_pv_total - 1))
                            npv[h] += 1

                        # ---- full (unmasked) ki blocks, two per psum ------
                        for pi, pair in enumerate(fpairs):
                            ps2, pt = {}, {}
                            for h in heads:
                                ps2[h] = pmm.tile([128, 1024], F32, tag="ps",
                                                  name=f"s{qj}_{h}_{pi}")
                            for j, ki in enumerate(pair):
                                for h in heads:
                                    base = (h % 2) * 64
                                    qsl = qkTb[t][qj][base:base + 64, :]
                                    ksl = qkTb[4 + t][ki // 4][
                                        base:base + 64,
                                        (ki % 4) * 128:(ki % 4 + 1) * 128]
                                    nc.tensor.matmul(
                                        ps2[h][:, j * 512:(j + 1) * 512],
                                        lhsT=ksl, rhs=qsl,
                                        start=True, stop=True)
                            for h in heads:
                                pt[h] = ptp.tile([128, 1024], DT_PV, tag="pt",
                                                 name=f"p{qj}_{h}_{pi}")
                                nc.scalar.activation(
                                    out=pt[h][:, :], in_=ps2[h][:, :],
                                    func=mybir.ActivationFunctionType.Exp)
                            for h in heads:
                                for j, ki in enumerate(pair):
                                    pv(h, ki, pt[h][:, j * 512:(j + 1) * 512])
                            drop_qkv(1)

                        # ---- diagonal ki blocks, fine-grained q ranges ----
                        # tile A: ki=4qj   q[0:512)   -> cols [0:512)
                        #         ki=4qj+1 q[128:512) -> cols [512:896)
                        # tile B: ki=4qj+2 q[256:512) -> cols [0:256)
                        #         ki=4qj+3 q[384:512) -> cols [256:384)
                        kd = 4 * qj
                        for h in heads:
                            base = (h % 2) * 64
                            qsl = qkTb[t][qj]
                            ksl4 = qkTb[4 + t][qj]

                            def smm(dst, ki, q0):
                                nc.tensor.matmul(
                                    dst,
                                    lhsT=ksl4[base:base + 64,
                                              (ki % 4) * 128:
                                              (ki % 4 + 1) * 128],
                                    rhs=qsl[base:base + 64, q0:512],
                                    start=True, stop=True)

                            psA = pmm.tile([128, 1024], F32, tag="ps",
                                           name=f"dA{qj}_{h}")
                            smm(psA[:, 0:512], kd, 0)
                            smm(psA[:, 512:896], kd + 1, 128)
                            ptA = ptp.tile([128, 1024], DT_PV, tag="pt",
                                           name=f"pA{qj}_{h}")
                            nc.scalar.activation(
                                out=ptA[:, 0:896], in_=psA[:, 0:896],
                                func=mybir.ActivationFunctionType.Exp)
                            nc.vector.tensor_mul(
                                ptA[:, 0:128], ptA[:, 0:128], tri_sb[:, :])
                            nc.vector.tensor_mul(
                                ptA[:, 512:640], ptA[:, 512:640],
                                tri_sb[:, :])
                            pv(h, kd, ptA[:, 0:512])
                            pv(h, kd + 1, ptA[:, 512:896], q0=128)

                            psB = pmm.tile([128, 512], F32, tag="ps",
                                           name=f"dB{qj}_{h}")
                            smm(psB[:, 0:256], kd + 2, 256)
                            smm(psB[:, 256:384], kd + 3, 384)
                            ptB = ptp.tile([128, 512], DT_PV, tag="pt",
                                           name=f"pB{qj}_{h}")
                            nc.scalar.activation(
                                out=ptB[:, 0:384], in_=psB[:, 0:384],
                                func=mybir.ActivationFunctionType.Exp)
                            nc.vector.tensor_mul(
                                ptB[:, 0:128], ptB[:, 0:128], tri_sb[:, :])
                            nc.vector.tensor_mul(
                                ptB[:, 256:384], ptB[:, 256:384],
                                tri_sb[:, :])
                            pv(h, kd + 2, ptB[:, 0:256], q0=256)
                            pv(h, kd + 3, ptB[:, 256:384], q0=384)
                            drop_qkv(1)

                        for h in heads:
                            osb = osp.tile([65, 512], F32, tag="osb",
                                           name=f"ob{qj}_{h}")
                            nc.vector.tensor_copy(osb[:, :], po[h][0:65, :])
                            # engines can't write partition h; DMA can
                            nc.sync.dma_start(den8[h:h + 1, :],
                                              osb[64:65, :])
                            osb_h[h] = osb

                        # PE filler for the exp-bound stretch: proj of the
                        # previous q-block rides this block's attention
                        if phases >= 3 and qj >= 1:
                            for e2, ls in PROJ_CHUNKS[hp]:
                                emit_proj(qj - 1, e2, ls)

                    # any QKV tiles the attention stream didn't absorb
                    drop_qkv(len(pending))

                    # ---- batched softmax normalization --------------------
                    inv8 = inp.tile([8, 512], F32, tag="inv8", name=f"iv{qj}")
                    nc.vector.reciprocal(inv8[:, :], den8[:, :])
                    for h in range(HLOC):
                        t = h // 2
                        base = (h % 2) * 64
                        invh = invp.tile([1, 512], F32, tag="invh",
                                         name=f"ivh{qj}_{h}")
                        nc.sync.dma_start(invh[:, :], inv8[h:h + 1, :])
                        rep = repp.tile([64, 512], F32, tag="rep",
                                        name=f"rp{qj}_{h}")
                        nc.gpsimd.partition_broadcast(
                            rep[:, :], invh[:, :], channels=64)
                        nc.vector.tensor_mul(
                            yTb[t][qj][base:base + 64, :],
                            osb_h[h][0:64, :], rep[:, :])

                if phases >= 3:
                    for chunk in PROJ_CHUNKS:
                        for e2, ls in chunk:
                            emit_proj(NLB - 1, e2, ls)
    return nc


def make_core_inputs(x, attn_mask, qkv_w, qkv_b, proj_w, patterns, cfg=None,
                     Lc=L):
    """Host-side shard prep: per-core input dicts for cores 0..7."""
    k = np.arange(128)[:, None]
    q = np.arange(128)[None, :]
    tri = (q >= k).astype(np.float32).astype(NP_IN)

    in_maps = []
    shared = {}
    for c in range(NCORES):
        b, hg = c // 2, c % 2
        if b not in shared:
            shared[b] = np.ascontiguousarray(
                np.asarray(x[b], np.float32).T).astype(NP_IN)
        key = ("w", hg)
        if key not in shared:
            rq = qkv_w[hg * 512:hg * 512 + 512, :] * SCALE
            rk = qkv_w[1024 + hg * 512:1024 + hg * 512 + 512, :]
            rv = qkv_w[2048 + hg * 512:2048 + hg * 512 + 512, :]
            w_qk_h = np.ascontiguousarray(
                np.concatenate([rq, rk], 0).T).astype(NP_IN)
            w_v_h = np.ascontiguousarray(rv.T).astype(NP_IN)
            bq = qkv_b[hg * 512:hg * 512 + 512] * SCALE
            bk = qkv_b[1024 + hg * 512:1024 + hg * 512 + 512]
            bqk_h = np.ascontiguousarray(
                np.concatenate([bq, bk]).reshape(8, 128).T).astype(np.float32)
            bv_h = np.ascontiguousarray(
                qkv_b[2048 + hg * 512:2048 + hg * 512 + 512].reshape(1, 512)
            ).astype(NP_IN)
            projT_h = np.ascontiguousarray(
                proj_w[:, hg * 512:hg * 512 + 512].T).astype(NP_IN)
            shared[key] = (w_qk_h, w_v_h, bqk_h, bv_h, projT_h)
        w_qk_h, w_v_h, bqk_h, bv_h, projT_h = shared[("w", hg)]
        in_maps.append({
            "xT": shared[b],
            "w_qk": w_qk_h,
            "w_v": w_v_h,
            "bqk": bqk_h,
            "bv": bv_h,
            "tri": tri,
            "projT": projT_h,
        })
    return in_maps


_NC_CACHE = {}
LAST_RESULTS = None
DEFAULT_CFG = None  # kept for test.py compat


def kernel(**inputs):
    x = np.asarray(inputs["x"], np.float32)
    attn_mask = np.asarray(inputs["attn_mask"])
    qkv_w = np.asarray(inputs["qkv_w"], np.float32)
    qkv_b = np.asarray(inputs["qkv_b"], np.float32)
    proj_w = np.asarray(inputs["proj_w"], np.float32)
    proj_b = np.asarray(inputs["proj_b"], np.float32)

    sched, patterns = schedule_from_mask(attn_mask, L)
    if not verify_causal(sched, patterns, L):
        raise ValueError("attn_mask is not block-causal tril; this kernel "
                         "is specialized for the causal mask")

    key = (L, tuple(tuple(r) for r in sched))
    if key not in _NC_CACHE:
        nc = build_nc(L, sched, len(patterns))
        if not nc.is_finalized():
            nc.finalize()
        _NC_CACHE[key] = nc
    nc = _NC_CACHE[key]

    in_maps = make_core_inputs(x, attn_mask, qkv_w, qkv_b, proj_w, patterns)
    res = run_bass_kernel_spmd(nc, in_maps, list(range(NCORES)))
    global LAST_RESULTS
    LAST_RESULTS = res

    out = np.empty((B, L, DIM), np.float32)
    for b in range(B):
        out[b] = (res.results[2 * b]["y"] + res.results[2 * b + 1]["y"]
                  + proj_b)
    return out


# revision 21
# speedup vs baseline: 1.0206x; 1.0206x over previous
"""Causal self-attention (B=4, L=2048, D=1024, H=16) on 8 Trainium2 NeuronCores.

Sharding: core c handles batch b = c//2 and head-group hg = c%2 (8 of 16 heads).
Each core computes its local QKV projection, causal flash-style attention for
its 8 heads, and a partial output projection against its 512 columns of
proj_w. The host sums the two partial outputs per batch and adds proj_b.

Device layouts (per core):
  xT    [1024, L]   x[b].T              (contraction dim d on partitions)
  w_qk  [1024,1024] qkv_w local q+k rows, transposed; q part pre-scaled by
                    HEAD_DIM**-0.5 (folded into weights+bias); fully
                    resident in SBUF
  qkTb  [128,512]x(8,4)  (q;k) features on partitions, one tile per l-block
  vaug  [L, 8*65]   v in token-major layout, one extra ones-column per head
                    (the ones column makes the PV matmul also produce the
                    softmax denominator as psum row 64)
  yTb   [128,512]x(4,4)  normalized attention output, per l-block
  projT [512, 1024] proj_w local columns, transposed

Softmax skips the max-subtraction (scores are O(+-10) here, far from fp32
overflow) so P = exp(S).

Perf structure (the attention inner loop is exp-paced on the Scalar engine;
everything else must hide under it):
  - diagonal ki-tiles computed at fine granularity: for ki = 4*qj+j the
    score matmul covers only q in [128j, 512), so scores/exp/PV shrink by
    ~25% on the diagonal and masking reduces to one [128,128] triangle
    multiply per tile (exact-causal, no wasted exp on masked halves)
  - QKV of l-block lb+1 and proj of lb-1 are emitted in chunks inside
    attention(lb)'s head loop so their PE work fills the exp-bound stretch
  - reciprocal batched to one [8,512] op per l-block (DVE iterative divide
    is ~6.4 cyc/elem, so per-head [1,512] reciprocals were ~3.3us each)
  - denominator rows cross partitions via DMA (engines can't address
    single partitions off 32-alignment)
"""

import os
import sys

import numpy as np

for _p in ("/opt/trn_rl_repo", "/root/.axon_site/_ro/trn_rl_repo"):
    if os.path.isdir(_p) and _p not in sys.path:
        sys.path.append(_p)

import ml_dtypes  # noqa: E402
import concourse.bass as bass  # noqa: E402
import concourse.tile as tile  # noqa: E402
from concourse import bacc, mybir  # noqa: E402
from concourse.bass_utils import run_bass_kernel_spmd  # noqa: E402

DIM = 1024
NUM_HEADS = 16
HEAD_DIM = 64
SCALE = HEAD_DIM**-0.5
B = 4
L = 2048
NCORES = 8
HLOC = 8  # heads per core

F32 = mybir.dt.float32
BF16 = mybir.dt.bfloat16

DT_IN = BF16   # x / weights
DT_QK = BF16   # q/k storage
DT_PV = BF16   # P / v_aug
DT_PROJ = BF16  # yT / projT

NP_IN = ml_dtypes.bfloat16


def schedule_from_mask(am, Lc):
    """Classify [128k x 512q] score blocks from attn_mask[q, k].

    Returns (sched, patterns): sched[qj] = list of (ki, pat_idx|None) blocks
    to compute; patterns = list of [128, 512] float32 0/1 arrays (k on
    partitions, q on free dim) for partially-masked blocks.
    """
    am = np.asarray(am) != 0
    sched, patterns, pat_ids = [], [], {}
    for qj in range(Lc // 512):
        row = []
        for ki in range(Lc // 128):
            blk = am[qj * 512:(qj + 1) * 512, ki * 128:(ki + 1) * 128]  # [q,k]
            if not blk.any():
                continue
            if blk.all():
                row.append((ki, None))
                continue
            pat = np.ascontiguousarray(blk.T).astype(np.float32)  # [k,q]
            key = pat.tobytes()
            if key not in pat_ids:
                pat_ids[key] = len(patterns)
                patterns.append(pat)
            row.append((ki, pat_ids[key]))
        sched.append(row)
    return sched, patterns


def verify_causal(sched, patterns, Lc):
    """Check the mask is exactly causal-tril at block granularity (the
    fine-grained diagonal codegen assumes it)."""
    k = np.arange(128)[:, None]
    for qj in range(Lc // 512):
        blocks = sched[qj]
        full = [ki for ki, pat in blocks if pat is None]
        diag = [(ki, pat) for ki, pat in blocks if pat is not None]
        if full != list(range(4 * qj)) or len(diag) != 4:
            return False
        for ki, pat in diag:
            j = ki - 4 * qj
            if j not in range(4):
                return False
            q = np.arange(512)[None, :]
            expect = (q >= 128 * j + k).astype(np.float32)
            if not np.array_equal(patterns[pat], expect):
                return False
    return True


def build_nc(Lc, sched, n_pat=0, cfg=None, nrep=1, phases=3):
    """Emit the per-core Bass/Tile program. Same program runs on all cores."""
    NLB = Lc // 512   # l-blocks (also q-blocks)
    NKT = Lc // 128   # k-tiles
    ND = DIM // 128   # contraction tiles for QKV

    nc = bacc.Bacc("TRN2", target_bir_lowering=False, debug=False)

    xT = nc.dram_tensor("xT", [DIM, Lc], DT_IN, kind="ExternalInput")
    w_qk = nc.dram_tensor("w_qk", [DIM, 1024], DT_IN, kind="ExternalInput")
    w_v = nc.dram_tensor("w_v", [DIM, 512], DT_IN, kind="ExternalInput")
    bqk = nc.dram_tensor("bqk", [128, 8], F32, kind="ExternalInput")
    bv = nc.dram_tensor("bv", [1, 512], DT_IN, kind="ExternalInput")
    tri_d = nc.dram_tensor("tri", [128, 128], DT_PV, kind="ExternalInput")
    projT = nc.dram_tensor("projT", [512, 1024], DT_PROJ, kind="ExternalInput")
    y = nc.dram_tensor("y", [Lc, 1024], F32, kind="ExternalOutput")

    with tile.TileContext(nc) as tc:
        import contextlib
        with contextlib.ExitStack() as ctx:
            sing = ctx.enter_context(tc.tile_pool(name="sing", bufs=1))

            qkTb = [[sing.tile([128, 512], DT_QK, tag=f"qkT{e}_{lb}",
                               name=f"qkT{e}_{lb}") for lb in range(NLB)]
                    for e in range(8)]
            vaug = [sing.tile([128, HLOC * 65], DT_PV, tag=f"vaug{t}",
                              name=f"vaug{t}") for t in range(NKT)]
            yTb = [[sing.tile([128, 512], DT_PROJ, tag=f"yT{f}_{lb}",
                              name=f"yT{f}_{lb}") for lb in range(NLB)]
                   for f in range(4)]
            projT_sb = [sing.tile([128, 1024], DT_PROJ, tag=f"pw{t}",
                                  name=f"pw{t}") for t in range(4)]
            wv_sb = [sing.tile([128, 512], DT_IN, tag=f"wv{t}", name=f"wv{t}")
                     for t in range(ND)]
            wqk_sb = [sing.tile([128, 1024], DT_IN, tag=f"wqk{t}",
                                name=f"wqk{t}") for t in range(ND)]
            bqk_sb = sing.tile([128, 8], F32, tag="bqk_sb", name="bqk_sb")
            bv_sb = sing.tile([1, 512], DT_IN, tag="bv_sb", name="bv_sb")
            tri_sb = sing.tile([128, 128], DT_PV, tag="tri", name="tri")
            ones_col = sing.tile([1, 128], DT_IN, tag="ones_col",
                                 name="ones_col")

            nc.vector.memset(ones_col[:, :], 1.0)
            for t in range(NKT):
                va = vaug[t].rearrange("p (h c) -> p h c", c=65)
                nc.vector.memset(va[:, :, 64:65], 1.0)

            # weights first: the first QKV matmuls gate on these
            for t in range(ND):
                nc.sync.dma_start(wqk_sb[t][:, :],
                                  w_qk[t * 128:(t + 1) * 128, :])
            for t in range(ND):
                nc.sync.dma_start(wv_sb[t][:, :], w_v[t * 128:(t + 1) * 128, :])
            nc.sync.dma_start(bqk_sb[:, :], bqk[:, :])
            nc.sync.dma_start(bv_sb[:, :], bv[:, :])
            nc.sync.dma_start(tri_sb[:, :], tri_d[:, :])
            for t in range(4):
                nc.sync.dma_start(projT_sb[t][:, :],
                                  projT[t * 128:(t + 1) * 128, :])

            xp = ctx.enter_context(tc.tile_pool(name="xp", bufs=18))
            ptp = ctx.enter_context(tc.tile_pool(name="ptp", bufs=6))
            osp = ctx.enter_context(tc.tile_pool(name="osp", bufs=12))
            dnp = ctx.enter_context(tc.tile_pool(name="dnp", bufs=2))
            inp = ctx.enter_context(tc.tile_pool(name="inp", bufs=2))
            invp = ctx.enter_context(tc.tile_pool(name="invp", bufs=6))
            repp = ctx.enter_context(tc.tile_pool(name="repp", bufs=4))
            outp = ctx.enter_context(tc.tile_pool(name="outp", bufs=4))
            pmm = ctx.enter_context(tc.tile_pool(name="pmm", bufs=3,
                                                 space="PSUM"))
            pov = ctx.enter_context(tc.tile_pool(name="pov", bufs=2,
                                                 space="PSUM"))

            def emit_x_dma(lb):
                xt = []
                for d in range(ND):
                    xd = xp.tile([128, 512], DT_IN, tag="xd",
                                 name=f"x{lb}_{d}")
                    nc.sync.dma_start(
                        xd[:, :],
                        xT[d * 128:(d + 1) * 128, lb * 512:lb * 512 + 512])
                    xt.append(xd)
                return xt

            def emit_qkv_qk(lb, e, xt):
                ps = pmm.tile([128, 512], F32, tag="ps", name=f"q{lb}_{e}")
                for d in range(ND):
                    nc.tensor.matmul(
                        ps[:, :],
                        lhsT=wqk_sb[d][:, e * 128:(e + 1) * 128],
                        rhs=xt[d][:, :],
                        start=(d == 0), stop=(d == ND - 1))
                nc.vector.tensor_scalar_add(
                    out=qkTb[e][lb][:, :], in0=ps[:, :],
                    scalar1=bqk_sb[:, e:e + 1])

            def emit_qkv_v(lb, ls, xt):
                lt = lb * 4 + ls
                ps = pmm.tile([128, 512], F32, tag="ps", name=f"v{lt}")
                for d in range(ND):
                    nc.tensor.matmul(
                        ps[:, :],
                        lhsT=xt[d][:, ls * 128:(ls + 1) * 128],
                        rhs=wv_sb[d][:, :],
                        start=(d == 0), stop=False)
                nc.tensor.matmul(
                    ps[:, :], lhsT=ones_col[:, :],
                    rhs=bv_sb[:, :], start=False, stop=True)
                dst = vaug[lt].rearrange("p (h c) -> p h c", c=65)[:, :, 0:64]
                src = ps.rearrange("p (h c) -> p h c", c=64)
                nc.vector.tensor_copy(dst, src)

            def emit_proj(qj, e2, ls):
                lr = qj * 512 + ls * 128
                # proj shares the pov pool's slots (same 1-bank size as po)
                # instead of the scores/qkv "ps" rotation, so the exp feed
                # never waits behind a proj drain
                ps = pov.tile([128, 512], F32, tag="po",
                              name=f"pj{qj}_{e2}_{ls}")
                for f in range(4):
                    nc.tensor.matmul(
                        ps[:, :],
                        lhsT=yTb[f][qj][:, ls * 128:(ls + 1) * 128],
                        rhs=projT_sb[f][:, e2 * 512:(e2 + 1) * 512],
                        start=(f == 0), stop=(f == 3))
                ob = outp.tile([128, 512], F32, tag="ob",
                               name=f"yo{qj}_{e2}_{ls}")
                nc.vector.tensor_copy(ob[:, :], ps[:, :])
                nc.sync.dma_start(
                    y[lr:lr + 128, e2 * 512:(e2 + 1) * 512], ob[:, :])

            PROJ_CHUNKS = [[(0, 0), (0, 1)], [(0, 2), (0, 3)],
                           [(1, 0), (1, 1)], [(1, 2), (1, 3)]]

            _ENGS = (mybir.EngineType.PE, mybir.EngineType.Activation,
                     mybir.EngineType.DVE, mybir.EngineType.Pool,
                     mybir.EngineType.SP)
            rep_ctx = (tc.For_i(0, nrep, 1, hint_engines=_ENGS) if nrep > 1
                       else contextlib.nullcontext())
            with rep_ctx:
                # l-block 0's QKV runs up front; later blocks are emitted in
                # chunks inside the previous block's attention
                xt_cur = emit_x_dma(0)
                for e in range(8):
                    emit_qkv_qk(0, e, xt_cur)
                for ls in range(4):
                    emit_qkv_v(0, ls, xt_cur)

                for qj in range(NLB):
                    if phases < 2:
                        break
                    xt_next = emit_x_dma(qj + 1) if qj + 1 < NLB else None
                    blocks = sched[qj]
                    full = [ki for ki, pat in blocks if pat is None]
                    fpairs = [full[i:i + 2] for i in range(0, len(full), 2)]
                    den8 = dnp.tile([8, 512], F32, tag="den8", name=f"dn{qj}")
                    osb_h = {}
                    # next l-block's QKV, dripped one psum tile at a time
                    # into the exp-paced attention stream (a burst of QKV
                    # matmuls here starves the Scalar engine; one ~2us tile
                    # per pair-iteration slots under the exp lookahead)
                    pending = []
                    if xt_next is not None:
                        pending = ([("qk", e) for e in range(8)]
                                   + [("v", ls) for ls in range(4)])

                    def drop_qkv(nmax=1):
                        for _ in range(nmax):
                            if not pending:
                                return
                            kind, idx = pending.pop(0)
                            if kind == "qk":
                                emit_qkv_qk(qj + 1, idx, xt_next)
                            else:
                                emit_qkv_v(qj + 1, idx, xt_next)
                    for hp in range(HLOC // 2):
                        t = hp
                        heads = (2 * hp, 2 * hp + 1)
                        po = {}
                        for h in heads:
                            po[h] = pov.tile([65, 512], F32, tag="po",
                                             name=f"o{qj}_{h}")
                        npv = {h: 0 for h in heads}
                        n_pv_total = len(full) + 4

                        def pv(h, ki, rhs_ap, q0=0):
                            nc.tensor.matmul(
                                po[h][:, q0:512],
                                lhsT=vaug[ki][:, h * 65:(h + 1) * 65],
                                rhs=rhs_ap,
                                start=(npv[h] == 0),
                                stop=(npv[h] == n_pv_total - 1))
                            npv[h] += 1

                        # ---- full (unmasked) ki blocks, two per psum ------
                        for pi, pair in enumerate(fpairs):
                            ps2, pt = {}, {}
                            for h in heads:
                                ps2[h] = pmm.tile([128, 1024], F32, tag="ps",
                                                  name=f"s{qj}_{h}_{pi}")
                            for j, ki in enumerate(pair):
                                for h in heads:
                                    base = (h % 2) * 64
                                    qsl = qkTb[t][qj][base:base + 64, :]
                                    ksl = qkTb[4 + t][ki // 4][
                                        base:base + 64,
                                        (ki % 4) * 128:(ki % 4 + 1) * 128]
                                    nc.tensor.matmul(
                                        ps2[h][:, j * 512:(j + 1) * 512],
                                        lhsT=ksl, rhs=qsl,
                                        start=True, stop=True)
                            for h in heads:
                                pt[h] = ptp.tile([128, 1024], DT_PV, tag="pt",
                                                 name=f"p{qj}_{h}_{pi}")
                                nc.scalar.activation(
                                    out=pt[h][:, :], in_=ps2[h][:, :],
                                    func=mybir.ActivationFunctionType.Exp)
                            for h in heads:
                                for j, ki in enumerate(pair):
                                    pv(h, ki, pt[h][:, j * 512:(j + 1) * 512])
                            drop_qkv(1)

                        # ---- diagonal ki blocks, fine-grained q ranges ----
                        # tile A: ki=4qj   q[0:512)   -> cols [0:512)
                        #         ki=4qj+1 q[128:512) -> cols [512:896)
                        # tile B: ki=4qj+2 q[256:512) -> cols [0:256)
                        #         ki=4qj+3 q[384:512) -> cols [256:384)
                        kd = 4 * qj
                        for h in heads:
                            base = (h % 2) * 64
                            qsl = qkTb[t][qj]
                            ksl4 = qkTb[4 + t][qj]

                            def smm(dst, ki, q0):
                                nc.tensor.matmul(
                                    dst,
                                    lhsT=ksl4[base:base + 64,
                                              (ki % 4) * 128:
                                              (ki % 4 + 1) * 128],
                                    rhs=qsl[base:base + 64, q0:512],
                                    start=True, stop=True)

                            psA = pmm.tile([128, 1024], F32, tag="ps",
                                           name=f"dA{qj}_{h}")
                            smm(psA[:, 0:512], kd, 0)
                            smm(psA[:, 512:896], kd + 1, 128)
                            ptA = ptp.tile([128, 1024], DT_PV, tag="pt",
                                           name=f"pA{qj}_{h}")
                            nc.scalar.activation(
                                out=ptA[:, 0:896], in_=psA[:, 0:896],
                                func=mybir.ActivationFunctionType.Exp)
                            nc.vector.tensor_mul(
                                ptA[:, 0:128], ptA[:, 0:128], tri_sb[:, :])
                            nc.vector.tensor_mul(
                                ptA[:, 512:640], ptA[:, 512:640],
                                tri_sb[:, :])
                            pv(h, kd, ptA[:, 0:512])
                            pv(h, kd + 1, ptA[:, 512:896], q0=128)

                            psB = pmm.tile([128, 512], F32, tag="ps",
                                           name=f"dB{qj}_{h}")
                            smm(psB[:, 0:256], kd + 2, 256)
                            smm(psB[:, 256:384], kd + 3, 384)
                            ptB = ptp.tile([128, 512], DT_PV, tag="pt",
                                           name=f"pB{qj}_{h}")
                            nc.scalar.activation(
                                out=ptB[:, 0:384], in_=psB[:, 0:384],
                                func=mybir.ActivationFunctionType.Exp)
                            nc.vector.tensor_mul(
                                ptB[:, 0:128], ptB[:, 0:128], tri_sb[:, :])
                            nc.vector.tensor_mul(
                                ptB[:, 256:384], ptB[:, 256:384],
                                tri_sb[:, :])
                            pv(h, kd + 2, ptB[:, 0:256], q0=256)
                            pv(h, kd + 3, ptB[:, 256:384], q0=384)
                            drop_qkv(1)

                        for h in heads:
                            osb = osp.tile([65, 512], F32, tag="osb",
                                           name=f"ob{qj}_{h}")
                            nc.vector.tensor_copy(osb[:, :], po[h][0:65, :])
                            # engines can't write partition h; DMA can
                            nc.sync.dma_start(den8[h:h + 1, :],
                                              osb[64:65, :])
                            osb_h[h] = osb

                        # PE filler for the exp-bound stretch: proj of the
                        # previous q-block rides this block's attention
                        if phases >= 3 and qj >= 1:
                            for e2, ls in PROJ_CHUNKS[hp]:
                                emit_proj(qj - 1, e2, ls)

                    # any QKV tiles the attention stream didn't absorb
                    drop_qkv(len(pending))

                    # ---- batched softmax normalization --------------------
                    inv8 = inp.tile([8, 512], F32, tag="inv8", name=f"iv{qj}")
                    nc.vector.reciprocal(inv8[:, :], den8[:, :])
                    for h in range(HLOC):
                        t = h // 2
                        base = (h % 2) * 64
                        invh = invp.tile([1, 512], F32, tag="invh",
                                         name=f"ivh{qj}_{h}")
                        nc.sync.dma_start(invh[:, :], inv8[h:h + 1, :])
                        rep = repp.tile([64, 512], F32, tag="rep",
                                        name=f"rp{qj}_{h}")
                        nc.gpsimd.partition_broadcast(
                            rep[:, :], invh[:, :], channels=64)
                        nc.vector.tensor_mul(
                            yTb[t][qj][base:base + 64, :],
                            osb_h[h][0:64, :], rep[:, :])

                if phases >= 3:
                    for chunk in PROJ_CHUNKS:
                        for e2, ls in chunk:
                            emit_proj(NLB - 1, e2, ls)
    return nc


def make_core_inputs(x, attn_mask, qkv_w, qkv_b, proj_w, patterns, cfg=None,
                     Lc=L):
    """Host-side shard prep: per-core input dicts for cores 0..7."""
    k = np.arange(128)[:, None]
    q = np.arange(128)[None, :]
    tri = (q >= k).astype(np.float32).astype(NP_IN)

    in_maps = []
    shared = {}
    for c in range(NCORES):
        b, hg = c // 2, c % 2
        if b not in shared:
            shared[b] = np.ascontiguousarray(
                np.asarray(x[b], np.float32).T).astype(NP_IN)
        key = ("w", hg)
        if key not in shared:
            rq = qkv_w[hg * 512:hg * 512 + 512, :] * SCALE
            rk = qkv_w[1024 + hg * 512:1024 + hg * 512 + 512, :]
            rv = qkv_w[2048 + hg * 512:2048 + hg * 512 + 512, :]
            w_qk_h = np.ascontiguousarray(
                np.concatenate([rq, rk], 0).T).astype(NP_IN)
            w_v_h = np.ascontiguousarray(rv.T).astype(NP_IN)
            bq = qkv_b[hg * 512:hg * 512 + 512] * SCALE
            bk = qkv_b[1024 + hg * 512:1024 + hg * 512 + 512]
            bqk_h = np.ascontiguousarray(
                np.concatenate([bq, bk]).reshape(8, 128).T).astype(np.float32)
            bv_h = np.ascontiguousarray(
                qkv_b[2048 + hg * 512:2048 + hg * 512 + 512].reshape(1, 512)
            ).astype(NP_IN)
            projT_h = np.ascontiguousarray(
                proj_w[:, hg * 512:hg * 512 + 512].T).astype(NP_IN)
            shared[key] = (w_qk_h, w_v_h, bqk_h, bv_h, projT_h)
        w_qk_h, w_v_h, bqk_h, bv_h, projT_h = shared[("w", hg)]
        in_maps.append({
            "xT": shared[b],
            "w_qk": w_qk_h,
            "w_v": w_v_h,
            "bqk": bqk_h,
            "bv": bv_h,
            "tri": tri,
            "projT": projT_h,
        })
    return in_maps


_NC_CACHE = {}
LAST_RESULTS = None
DEFAULT_CFG = None  # kept for test.py compat


def kernel(**inputs):
    x = np.asarray(inputs["x"], np.float32)
    attn_mask = np.asarray(inputs["attn_mask"])
    qkv_w = np.asarray(inputs["qkv_w"], np.float32)
    qkv_b = np.asarray(inputs["qkv_b"], np.float32)
    proj_w = np.asarray(inputs["proj_w"], np.float32)
    proj_b = np.asarray(inputs["proj_b"], np.float32)

    sched, patterns = schedule_from_mask(attn_mask, L)
    if not verify_causal(sched, patterns, L):
        raise ValueError("attn_mask is not block-causal tril; this kernel "
                         "is specialized for the causal mask")

    key = (L, tuple(tuple(r) for r in sched))
    if key not in _NC_CACHE:
        nc = build_nc(L, sched, len(patterns))
        if not nc.is_finalized():
            nc.finalize()
        _NC_CACHE[key] = nc
    nc = _NC_CACHE[key]

    in_maps = make_core_inputs(x, attn_mask, qkv_w, qkv_b, proj_w, patterns)
    res = run_bass_kernel_spmd(nc, in_maps, list(range(NCORES)))
    global LAST_RESULTS
    LAST_RESULTS = res

    out = np.empty((B, L, DIM), np.float32)
    for b in range(B):
        out[b] = (res.results[2 * b]["y"] + res.results[2 * b + 1]["y"]
                  + proj_b)
    return out
